# revision 33
# baseline (speedup 1.0000x reference)
"""Trainium2 Bass kernel: cosine-similarity softmin retrieval (DSDM).

reference:  qn = q/||q||; an = a/||a||; sims = qn @ an^T            [B, N]
            w = softmax(10*sims) over N  (softmin of (1-sims)/0.1)
            out = (w @ A)                                           [B, D]

v3 strategy (8 NeuronCores, flash-attention-style split over N):
  - addresses [200000, 512] sharded row-wise, 25000 rows/core.
  - per core the shard streams once in 512-row "quads" (49 of them):
      * one SWDGE cast-DMA per quad, row-permuted (p t) d -> p t d:
        partition p holds rows 4p..4p+3 => 8KB contiguous descriptors.
        The permutation cancels between sims and retrieval.
      * row norms ss = sum(a^2): DVE affine_mul_reduce / ACT Square
        per NORM_PAT; inv = 10/||a|| = exp(-0.5*ln(ss+eps)+ln10) (ACT)
      * A^T via PE transposes (2 tiles per PSUM bank -> one DVE copy
        per 2 tiles)
      * sims computed TRANSPOSED: s^T[j, b] with lhsT = A^T chunks,
        rhs = qn^T (j is the partition dim, so the per-row 10/||a||
        scale rides the ACT exp's per-partition scale operand for free)
      * w^T = Exp(s^T * inv - 10) per tile on ACT -> [128, 4, 64] bf16
        (fixed shift: cos<=1 so logit-10 <= 0; no running max needed)
      * retrieval: acc[64, 512] += w^T.T @ A in PSUM across all tiles
      * lsum: wsum[128, 4, 64] += w^T on GPSIMD; ones-matmul at end
  - host: out = sum_c acc_c / sum_c l_c   (gather/unshard + tiny divide)

Padding: per-core 25000 rows = 48 full quads + 424 rows (partitions
0..105 of quad 48); partitions 106..127 are zeroed and contribute
exactly exp(-10) each to lsum, subtracted on the host.
"""

import math
import os
from collections import OrderedDict

import numpy as np

import concourse.bass as bass
import concourse.tile as tile
from concourse import bacc, mybir
from concourse.bass_utils import run_bass_kernel_spmd
from concourse.masks import make_identity

DT = mybir.dt
AF = mybir.ActivationFunctionType

B = 64
D = 512
N_FULL = 200000
NCORES = 8
NPC = N_FULL // NCORES  # 25000
P = 128
QROWS = 4 * P  # rows per quad
LN10 = math.log(10.0)

# engine assignment knobs (v=DVE, a=ACT), one char per tile-in-quad
NORM_PAT = os.environ.get("KERNEL_NORM_PAT", "vvaa")
DMA_AHEAD = int(os.environ.get("KERNEL_DMA_AHEAD", "5"))
WSUM_MODE = os.environ.get("KERNEL_WSUM", "gpsimd")  # "pe" or "gpsimd"
ABUFS = int(os.environ.get("KERNEL_ABUFS", "10"))
ATBUFS = int(os.environ.get("KERNEL_ATBUFS", "4"))
LDW_OPT = os.environ.get("KERNEL_LDW_OPT", "0") == "1"

LAST_RESULTS = None  # test harness reads exec_time_ns from here


def _patch_act_tables():
    """Prefer the combined natural_log_exp set so Ln/Exp/Square/Copy share
    one ACT table load instead of thrashing 2 loads per quad (~2.7us each)."""
    if getattr(bacc.get_activation_tables, "_patched", False):
        return
    orig = bacc.get_activation_tables

    keep = {AF.Ln, AF.Exp, AF.Square, AF.Copy}

    def patched(arch):
        tabs = orig(arch)
        out = OrderedDict()
        for k, fns in tabs.items():
            if k == "natural_log_exp_and_others":
                out[k] = fns
            else:
                out[k] = {f for f in fns if f not in keep}
        return out

    patched._patched = True
    bacc.get_activation_tables = patched


def _patch_ldw_opt():
    """Opt-in: flip walrus --enable-ldw-opt to true (experiment knob)."""
    from concourse import bass_utils

    if getattr(bass_utils.run_command, "_ldw_patched", False):
        return
    orig = bass_utils.run_command

    def patched(cmd, *a, **kw):
        cmd = [c.replace("--enable-ldw-opt=false", "--enable-ldw-opt=true")
               if isinstance(c, str) else c for c in cmd]
        return orig(cmd, *a, **kw)

    patched._ldw_patched = True
    bass_utils.run_command = patched


def _build(npc=NPC):
    _patch_act_tables()
    if LDW_OPT:
        _patch_ldw_opt()
    assert npc % 4 == 0 and npc >= QROWS
    nquads = (npc + QROWS - 1) // QROWS
    ntiles = 4 * nquads
    # the last quad re-reads the final QROWS rows (full-width DMA); the
    # o4 partitions that overlap the previous quad get exp bias -40 so
    # their duplicated weights are ~e^-30 smaller: negligible.
    o4 = (nquads * QROWS - npc) // 4  # 22 for npc=25000

    nc = bacc.Bacc("TRN2")
    q_d = nc.dram_tensor("query", [B, D], DT.float32, kind="ExternalInput")
    a_d = nc.dram_tensor("addresses", [npc, D], DT.float32, kind="ExternalInput")
    acc_d = nc.dram_tensor("acc", [2 * B, D], DT.float32, kind="ExternalOutput")
    lsum_d = nc.dram_tensor("lsum", [B, 1], DT.float32, kind="ExternalOutput")

    with tile.TileContext(nc) as tc:
        with (
            tc.tile_pool(name="const", bufs=1) as const,
            tc.tile_pool(name="slab", bufs=ABUFS) as slab_pool,
            tc.tile_pool(name="at", bufs=ATBUFS) as at_pool,
            tc.tile_pool(name="wt", bufs=4) as wt_pool,
            tc.tile_pool(name="small", bufs=4) as small,
            tc.tile_pool(name="ps_at", bufs=4, space="PSUM") as ps_at,
            tc.tile_pool(name="ps_s", bufs=2, space="PSUM") as ps_s,
            tc.tile_pool(name="ps_acc", bufs=1, space="PSUM") as ps_acc,
            tc.tile_pool(name="ps_l", bufs=1, space="PSUM") as ps_l,
        ):
            ident = const.tile([P, P], DT.bfloat16)
            make_identity(nc, ident)
            bias_exp = const.tile([P, 1], DT.float32)
            nc.vector.memset(bias_exp, -10.0)
            bias_last = const.tile([P, 1], DT.float32)
            nc.vector.memset(bias_last, -10.0)
            if o4:
                # bias_last[p] = p - o4 >= 0 ? -10 : -40
                nc.gpsimd.affine_select(
                    out=bias_last, in_=bias_last,
                    compare_op=mybir.AluOpType.is_ge, fill=-40.0,
                    base=-o4, pattern=[[0, 1]], channel_multiplier=1)
            ones = const.tile([P, 1], DT.bfloat16)
            nc.vector.memset(ones, 1.0)
            onesf = const.tile([P, 1], DT.float32)
            nc.vector.memset(onesf, 1.0)
            eps12 = const.tile([P, 1], DT.float32)
            nc.vector.memset(eps12, 1e-12)
            ln10b = const.tile([P, 1], DT.float32)
            nc.vector.memset(ln10b, LN10)
            wsum4 = const.tile([P, 4, B], DT.float32)
            nc.vector.memset(wsum4, 0.0)

            # ---- query preprocessing: qn^T bf16 chunks [128d, 4c, 64b] ----
            q_sb = const.tile([B, D], DT.float32)
            nc.sync.dma_start(out=q_sb, in_=q_d[:, :])
            qsq = const.tile([B, D], DT.float32)
            ssq = const.tile([B, 1], DT.float32)
            nc.scalar.activation(qsq, q_sb, AF.Square, accum_out=ssq)
            lnq = const.tile([B, 1], DT.float32)
            nc.scalar.activation(lnq, ssq, AF.Ln, bias=eps12[:B])
            invq = const.tile([B, 1], DT.float32)
            nc.scalar.activation(invq, lnq, AF.Exp, scale=-0.5)
            qn = const.tile([B, D], DT.bfloat16)
            nc.vector.tensor_scalar_mul(out=qn, in0=q_sb, scalar1=invq)
            qnT = const.tile([P, 4, B], DT.bfloat16)
            qt_ps = ps_l.tile([P, 4, B], DT.bfloat16, tag="l")
            for c in range(4):
                nc.tensor.transpose(qt_ps[:, c, :], qn[:, c * P:(c + 1) * P],
                                    ident[:B, :B])
            nc.scalar.copy(qnT, qt_ps)

            # [2*64, 512]: halves are independent col-group accumulation
            # chains (tiles of even/odd t), summed on the host
            acc_ps = ps_acc.tile([2 * B, D], DT.float32)
            if WSUM_MODE == "pe":
                l_ps = ps_l.tile([B, 1], DT.float32, tag="l")

            def norm_op(eng, sq, ss_col, a_t):
                if eng == "v":
                    nc.vector.affine_mul_reduce(
                        out=sq, accum_out=ss_col, in0=a_t, in1=a_t,
                        scale=1.0, bias=0.0)
                else:
                    nc.scalar.activation(sq, a_t, AF.Square, accum_out=ss_col)

            # ---- main streaming loop over quads ----
            # loads are emitted DMA_AHEAD quads early so the SWDGE issue
            # (gpsimd queue) isn't gated behind the same quad's wsum add
            a_slabs = {}

            def emit_load(qd):
                a_sl = slab_pool.tile([P, 4, D], DT.bfloat16)
                r0 = min(qd * QROWS, npc - QROWS)
                nc.gpsimd.dma_start(
                    out=a_sl,
                    in_=a_d[r0:r0 + QROWS, :].rearrange(
                        "(p t) d -> p t d", p=P))
                a_slabs[qd] = a_sl

            # norms for quad qd+1 are emitted at the end of iteration qd
            # (one quad early) so the exps never wait on them: by exp time
            # inv is already computed, and the ACT queue serves exps first.
            inv_cache = {}

            def emit_norms(qd):
                a_sl = a_slabs[qd]
                ss = small.tile([P, 4], DT.float32, tag="ss")
                for t in range(4):
                    sq = small.tile([P, D], DT.bfloat16, tag="sq")
                    norm_op(NORM_PAT[t], sq, ss[:, t:t + 1], a_sl[:, t, :])
                lns = small.tile([P, 4], DT.float32, tag="lns")
                nc.scalar.activation(lns, ss, AF.Ln, bias=eps12)
                inv = small.tile([P, 4], DT.float32, tag="inv")
                nc.scalar.activation(inv, lns, AF.Exp, scale=-0.5, bias=ln10b)
                inv_cache[qd] = inv

            for qd in range(min(DMA_AHEAD + 1, nquads)):
                emit_load(qd)
            emit_norms(0)
            for qd in range(nquads):
                a_sl = a_slabs.pop(qd)

                # A^T chunks: [d, tt, c, j], two tiles per PSUM bank.
                # Emitted before the norms so the DVE queue serves the
                # critical-path copies first.
                at_sb = []
                for pair in range(2):
                    at_ps2 = ps_at.tile([P, 2, 4, P], DT.bfloat16)
                    for tt in range(2):
                        t = 2 * pair + tt
                        for c in range(4):
                            nc.tensor.transpose(
                                at_ps2[:, tt, c, :],
                                a_sl[:, t, c * P:(c + 1) * P], ident)
                    at_sb2 = at_pool.tile([P, 2, 4, P], DT.bfloat16)
                    nc.vector.tensor_copy(at_sb2, at_ps2)
                    at_sb.append(at_sb2)

                # sims transposed: s^T[j, b] accumulated over 4 d-chunks
                s_q = ps_s.tile([P, 4, B], DT.float32, tag="s")
                for t in range(4):
                    pair, tt = divmod(t, 2)
                    for c in range(4):
                        nc.tensor.matmul(
                            s_q[:, t, :], lhsT=at_sb[pair][:, tt, c, :],
                            rhs=qnT[:, c, :], start=(c == 0), stop=(c == 3))

                # w^T = exp(s^T * inv - 10)   [128, 4, 64] bf16
                inv = inv_cache.pop(qd)
                wt_q = wt_pool.tile([P, 4, B], DT.bfloat16)
                bias_q = bias_last if qd == nquads - 1 else bias_exp
                for t in range(4):
                    nc.scalar.activation(
                        wt_q[:, t, :], s_q[:, t, :], AF.Exp,
                        bias=bias_q, scale=inv[:, t:t + 1])

                # retrieval: acc += w^T.T @ A. Tiles of even/odd t go to
                # separate PE column groups (M=64 each) and run concurrently.
                for t in range(4):
                    g = t % 2
                    nc.tensor.matmul(
                        acc_ps[g * B:(g + 1) * B, :],
                        lhsT=wt_q[:, t, :], rhs=a_sl[:, t, :],
                        start=(qd == 0 and t < 2),
                        stop=(qd == nquads - 1 and t >= 2))
                    if WSUM_MODE == "pe":
                        nc.tensor.matmul(
                            l_ps, lhsT=wt_q[:, t, :], rhs=ones,
                            start=(qd == 0 and t == 0),
                            stop=(qd == nquads - 1 and t == 3))
                if WSUM_MODE == "gpsimd":
                    nc.gpsimd.tensor_add(wsum4, wsum4, wt_q)
                if qd + 1 < nquads:
                    emit_norms(qd + 1)
                if qd + DMA_AHEAD + 1 < nquads:
                    emit_load(qd + DMA_AHEAD + 1)

            # ---- epilogue: normalizer + writeback ----
            if WSUM_MODE == "gpsimd":
                l_ps = ps_l.tile([B, 1], DT.float32, tag="l")
                for t in range(4):
                    nc.tensor.matmul(l_ps, lhsT=wsum4[:, t, :], rhs=onesf,
                                     start=(t == 0), stop=(t == 3))
            acc_sb = const.tile([2 * B, D], DT.float32)
            nc.vector.tensor_copy(acc_sb, acc_ps)
            l_sb = const.tile([B, 1], DT.float32)
            nc.vector.tensor_copy(l_sb, l_ps)
            nc.sync.dma_start(out=acc_d[:, :], in_=acc_sb)
            nc.sync.dma_start(out=lsum_d[:, :], in_=l_sb)

    nc.finalize()
    return nc


_NC_CACHE = {}


def _get_nc(npc=NPC):
    if npc not in _NC_CACHE:
        _NC_CACHE[npc] = _build(npc)
    return _NC_CACHE[npc]


def kernel(query, addresses):
    global LAST_RESULTS
    query = np.ascontiguousarray(np.asarray(query), dtype=np.float32)
    addresses = np.ascontiguousarray(np.asarray(addresses), dtype=np.float32)
    n = addresses.shape[0]
    npc = n // NCORES
    assert npc * NCORES == n
    nc = _get_nc(npc)
    in_maps = [
        {"query": query, "addresses": addresses[c * npc:(c + 1) * npc]}
        for c in range(NCORES)
    ]
    res = run_bass_kernel_spmd(nc, in_maps, core_ids=list(range(NCORES)))
    LAST_RESULTS = res
    acc = np.zeros((B, D), np.float64)
    l = np.zeros((B, 1), np.float64)
    for r in res.results:
        a2 = r["acc"].astype(np.float64)
        acc += a2[:B] + a2[B:]
        l += r["lsum"].astype(np.float64)
    return (acc / l).astype(np.float32)
